# revision 20
# baseline (speedup 1.0000x reference)
"""Multi-head attention (B=2,T=2048,D=1024,H=16,DK=64, causal, RoPE) on 8 TRN2 cores.

Sharding: data-parallel over batch (2) x tensor-parallel over heads (16 -> 4 per
core). core = 4*b + g handles batch b, heads [4g..4g+3]. RoPE tables replicated.
Host pre-transposes x and the projection weights, and permutes the q/k head dims
so each RoPE pair partner sits 16 partitions away inside a 32-partition
quadrant (DVE stream_shuffle reach). Each core returns a partial output
projection (bf16); the host sums the 4 head-group partials per batch in fp32
and adds the output bias.

v3 schedule notes:
- q/k biases are rank-1 matmuls appended to the PSUM accumulation chains;
  q/k/v PSUM evacuations run on ACT (idle during phase 1).
- RoPE pair-swap via DVE stream_shuffle (mask i^16) instead of SBUF-SBUF
  DMAs: the HWDGE generator (~630ns/DMA, shared by the sync+scalar queues)
  was saturating. RoPE runs per 512-chunk, K before Q, so phase 2 starts
  as soon as the last chunk's K is rotated.
- phase 2 emits the diagonal score pairs first so their exp + causal mask
  complete while the remaining pairs' scores stream; PV never waits.
- the output projection of chunk j is deferred into chunk j+1's score
  sections (2 tiles under dt=0, 2 under dt=1), filling the PE stall while
  ACT exps gate the scs WAR. q-chunk order (1, 0, 2, 3) gives the
  post-phase-1 boundary more PE work to hide the first exps.
- DMA queues: early weights on sync, late weights + tables on gpsimd
  (SWDGE), xt on scalar, out stores on scalar (free in phase 2).
"""

import sys

for _p in ("/opt/trn_rl_repo", "/root/.axon_site/_ro/trn_rl_repo"):
    if _p not in sys.path:
        sys.path.append(_p)

import numpy as np

from concourse import bacc, tile, mybir
import concourse.bass as bass
from concourse.bass2jax import _bass_exec_p, install_neuronx_cc_hook

B, T, D, H, DK = 2, 2048, 1024, 16, 64
G = 4          # heads per core
DSH = G * DK   # 256 sharded head dims per core
NCORES = 8
KT = D // 128  # 8 contraction tiles for projections
NTT = T // 128  # 16 row tiles
NCH = T // 512  # 4 column chunks
F32 = mybir.dt.float32
F32R = mybir.dt.float32r
BF16 = mybir.dt.bfloat16

SHUF_MASK = [(i ^ 16) for i in range(32)]  # RoPE pair partner, per quadrant

_CACHE = {}


def _build_bass():
    nc = bacc.Bacc("TRN2", target_bir_lowering=False, debug=False)

    xT = nc.dram_tensor("xT", [D, T], BF16, kind="ExternalInput").ap()
    wqT = nc.dram_tensor("wqT", [128, KT * DSH], BF16, kind="ExternalInput").ap()
    wkT = nc.dram_tensor("wkT", [128, KT * DSH], BF16, kind="ExternalInput").ap()
    wvT = nc.dram_tensor("wvT", [128, KT * DSH], BF16, kind="ExternalInput").ap()
    woT = nc.dram_tensor("woT", [128, 2 * D], F32, kind="ExternalInput").ap()
    bqkt = nc.dram_tensor("bqkt", [1, 512], F32, kind="ExternalInput").ap()
    bv = nc.dram_tensor("bv", [1, DSH], F32, kind="ExternalInput").ap()
    cc = nc.dram_tensor("cc", [128, T], BF16, kind="ExternalInput").ap()
    ss = nc.dram_tensor("ss", [128, T], BF16, kind="ExternalInput").ap()
    m01 = nc.dram_tensor("m01", [128, 128], BF16, kind="ExternalInput").ap()
    ones = nc.dram_tensor("ones", [1, 512], F32, kind="ExternalInput").ap()
    out = nc.dram_tensor("out", [T, D], BF16, kind="ExternalOutput").ap()

    def r(ap):  # fp32 storage -> fp32r matmul operand
        return ap.bitcast(F32R)

    with tile.TileContext(nc) as tc:
        with (
            tc.tile_pool(name="const", bufs=1) as const,
            tc.tile_pool(name="persist", bufs=1) as persist,
            tc.tile_pool(name="xt", bufs=3) as xtp,
            tc.tile_pool(name="rope", bufs=2) as ropep,
            tc.tile_pool(name="attn", bufs=2) as attnp,
            tc.tile_pool(name="epi", bufs=2) as epip,
        ):
            # ---- resident tensors ----
            # early weight slices (k0-3) on sync/HWDGE: needed first
            wq_sb = const.tile([128, KT, DSH], BF16)
            wk_sb = const.tile([128, KT, DSH], BF16)
            wv_sb = const.tile([128, KT, DSH], BF16)

            def wslice(w_sb, w_dram, kk, eng, nk=2):
                sl = slice(kk * DSH, (kk + nk) * DSH)
                wf = w_sb.rearrange("p k n -> p (k n)")
                eng.dma_start(out=wf[:, sl], in_=w_dram[:, sl])

            for kk in (0, 2):
                for w_sb, w_dram in ((wk_sb, wkT), (wq_sb, wqT), (wv_sb, wvT)):
                    wslice(w_sb, w_dram, kk, nc.sync)
            # late weights + tables on gpsimd/SWDGE, in need order
            for kk in (4, 6):
                for w_sb, w_dram in ((wq_sb, wqT), (wk_sb, wkT), (wv_sb, wvT)):
                    wslice(w_sb, w_dram, kk, nc.gpsimd)
            cc_sb = const.tile([128, T], BF16)
            ss_sb = const.tile([128, T], BF16)
            nc.gpsimd.dma_start(out=cc_sb, in_=cc)
            nc.gpsimd.dma_start(out=ss_sb, in_=ss)
            bqk_sb = const.tile([1, 512], F32R)
            nc.gpsimd.dma_start(out=bqk_sb, in_=bqkt.bitcast(F32R))
            ones_sb = const.tile([1, 512], F32R)
            nc.gpsimd.dma_start(out=ones_sb, in_=ones.bitcast(F32R))
            bv_sb = const.tile([1, DSH], F32R)
            nc.gpsimd.dma_start(out=bv_sb, in_=bv.bitcast(F32R))
            m01_sb = const.tile([128, 128], BF16)
            nc.gpsimd.dma_start(out=m01_sb, in_=m01)
            wo_sb = const.tile([128, 2, D], F32R)
            nc.gpsimd.dma_start(out=wo_sb.rearrange("p k n -> p (k n)"), in_=woT.bitcast(F32R))

            qT_sb = persist.tile([128, 2, T], BF16)   # [d-tile, t], heads 2*dt+{0,1}
            kT_sb = persist.tile([128, 2, T], BF16)
            v1_sb = persist.tile([128, G, NTT, 65], BF16)  # [s, head, s-tile, d|1]
            # only the ones-column needs init (softmax denominators); cols
            # 0:64 are fully overwritten by the v evacuations
            nc.vector.memset(v1_sb[:, :, :, 64:65], 1.0)
            ctxT_sb = persist.tile([128, 2, T], F32R)

            # ---- phase 1: projections + RoPE, one 512-wide t-chunk at a time ----
            # PSUM: qp/kp hold both head-tiles, each dt-half exactly one bank
            # (an accumulation chain must own its whole bank: start=True
            # clears has_written bank-wide); vp tiles are bank-padded.
            with tc.tile_pool(name="ps1", bufs=1, space="PSUM") as ps1:
                raw = {}
                raw[1] = ropep.tile([128, 2, T], BF16, tag="rawk", name="rawk", bufs=1)
                raw[0] = ropep.tile([128, 2, T], BF16, tag="rawq", name="rawq", bufs=1)
                for tch in range(NCH):
                    tsl = slice(512 * tch, 512 * tch + 512)
                    qp = ps1.tile([128, 2, 512], F32, tag="qp", name="qp")
                    kp = ps1.tile([128, 2, 512], F32, tag="kp", name="kp")
                    vp = [ps1.tile([128, 256], F32, tag=f"vp{i}", name=f"vp{i}")
                          for i in range(4)]
                    for k in range(KT):
                        xt = xtp.tile([128, 512], BF16, tag="xt")
                        nc.scalar.dma_start(out=xt, in_=xT[128 * k : 128 * k + 128, tsl])
                        for dt in range(2):
                            dsl = slice(128 * dt, 128 * dt + 128)
                            nc.tensor.matmul(kp[:, dt, :], wk_sb[:, k, dsl], xt,
                                             start=(k == 0), stop=False)
                            nc.tensor.matmul(qp[:, dt, :], wq_sb[:, k, dsl], xt,
                                             start=(k == 0), stop=False)
                        for tt in range(4):
                            nc.tensor.matmul(
                                vp[tt],
                                xt[:, 128 * tt : 128 * tt + 128],
                                wv_sb[:, k, :],
                                start=(k == 0), stop=False)
                    # biases as trailing rank-1 updates on the PE
                    for dt in range(2):
                        nc.tensor.matmul(kp[:, dt, :], bqk_sb[:, 256 + 128 * dt : 384 + 128 * dt],
                                         ones_sb, start=False, stop=True)
                        nc.tensor.matmul(qp[:, dt, :], bqk_sb[:, 128 * dt : 128 * dt + 128],
                                         ones_sb, start=False, stop=True)
                    for tt in range(4):  # + bv broadcast along t (rank-1 matmul)
                        nc.tensor.matmul(vp[tt], ones_sb[:, 0:128], bv_sb,
                                         start=False, stop=True)
                    # evacuations: k + v on ACT (k first: RoPE and the next
                    # chunk's kp matmuls need it), q on DVE before its RoPE
                    # use. On the last chunk everything runs on ACT in qp, kp,
                    # v order: phase 2's first scores wait on the qp/kp banks
                    # (PSUM reuse) while RoPE of chunk 3 is only needed by j=3.
                    def vevac(eng):
                        for tt in range(4):
                            eng.copy(out=v1_sb[:, :, 4 * tch + tt, 0:64], in_=vp[tt])
                    if tch == NCH - 1:
                        nc.scalar.copy(out=raw[0][:, :, tsl], in_=qp)
                        nc.scalar.copy(out=raw[1][:, :, tsl], in_=kp)
                        vevac(nc.scalar)
                    else:
                        nc.scalar.copy(out=raw[1][:, :, tsl], in_=kp)
                        vevac(nc.scalar)
                        nc.vector.tensor_copy(raw[0][:, :, tsl], qp)
                    # RoPE for this chunk: pair-swap via stream_shuffle, then
                    # dst = raw*cc + swap*ss on DVE (bf16 2x mode). K before Q.
                    for which, dst in ((1, kT_sb), (0, qT_sb)):
                        for dt in range(2):
                            rf = raw[which][:, dt, tsl]
                            swp = ropep.tile([128, 512], BF16, tag="swp", bufs=2)
                            nc.vector.stream_shuffle(swp, rf, SHUF_MASK)
                            t1 = ropep.tile([128, 512], BF16, tag="t1", bufs=2)
                            t2 = ropep.tile([128, 512], BF16, tag="t2", bufs=2)
                            nc.vector.tensor_mul(t1, rf, cc_sb[:, tsl])
                            nc.vector.tensor_mul(t2, swp, ss_sb[:, tsl])
                            nc.vector.tensor_add(dst[:, dt, tsl], t1, t2)

            # ---- phase 2+3: causal attention (transposed scores) with the
            # output projection interleaved per 512-wide q-chunk ----
            # Head pairs (2*dt, 2*dt+1): the odd head's q/k rows live at
            # partition 64, so its score matmuls land in PE row-groups 2-3 and
            # run concurrently with the even head's.
            with tc.tile_pool(name="ps2", bufs=1, space="PSUM") as ps2:

                def emit_score_pair(j, dt, p2, ats, scs, npairs):
                    # last pair holds diagonal s-tiles 4j+2/4j+3 whose
                    # q-columns < 256 are fully masked: skip them
                    co = 256 if p2 == npairs - 1 else 0
                    for i in range(2):
                        st = 2 * p2 + i
                        for hh in range(2):  # adjacent mms pack rows 0-63/64-127
                            rsl = slice(64 * hh, 64 * hh + 64)
                            nc.tensor.matmul(
                                scs[hh][:, 512 * i + co : 512 * i + 512],
                                kT_sb[rsl, dt, 128 * st : 128 * st + 128],
                                qT_sb[rsl, dt, 512 * j + co : 512 * j + 512],
                                start=True, stop=True)
                    for hh in range(2):
                        sc_v = scs[hh].rearrange("p (i c) -> p i c", i=2)[:, :, co:]
                        nc.scalar.activation(
                            out=ats[hh][:, 2 * p2 : 2 * p2 + 2, co:], in_=sc_v,
                            func=mybir.ActivationFunctionType.Exp, scale=0.125)

                def emit_mask(j, at, cols):
                    # causal fixup: diagonal block c sits at free offset
                    # (4j+c)*512 + 128c (stride 640); mask the given pair of
                    # blocks with one strided multiply by m01
                    c0 = cols[0]
                    base = at[:, 4 * j + c0, 128 * c0 : 128 * c0 + 128]
                    diag_ap = bass.AP(
                        tensor=base.tensor, offset=base.offset,
                        ap=[list(base.ap[0]), [640, len(cols)], [1, 128]])
                    m01_b = bass.AP(
                        tensor=m01_sb.tensor, offset=m01_sb.offset,
                        ap=[list(m01_sb.ap[0]), [0, len(cols)], [1, 128]])
                    nc.vector.tensor_mul(diag_ap, diag_ap, m01_b)

                def emit_pv(j, dt, hh, at, nst, st_order):
                    h = 2 * dt + hh
                    ct = ps2.tile([65, 512], F32, tag=f"ct{hh}", name=f"ct{hh}", bufs=1)
                    for n, st in enumerate(st_order):
                        # diagonal s-tiles: columns < 128c are fully masked --
                        # exclude them from the PV matmul
                        c = max(st - 4 * j, 0)
                        nc.tensor.matmul(
                            ct[:, 128 * c :], v1_sb[:, h, st, :],
                            at[:, st, 128 * c :],
                            start=(n == 0), stop=(n == nst - 1))
                    return ct

                def emit_normalize(j, dt, hh, ct, tail=False):
                    qsl = slice(512 * j, 512 * j + 512)
                    rr = epip.tile([1, 512], F32, tag="rr")
                    nc.vector.reciprocal(rr, ct[64:65, :])
                    rb = epip.tile([64, 512], F32, tag="rb")
                    nc.gpsimd.partition_broadcast(rb, rr)
                    if hh == 0:
                        nc.vector.tensor_mul(ctxT_sb[0:64, dt, qsl], ct[0:64, :], rb)
                    else:
                        stg = epip.tile([64, 512], F32R, tag="stg")
                        nc.vector.tensor_mul(stg, ct[0:64, :], rb)
                        # the scalar queue is idle at the tail; sync's SEQ may
                        # still be blocked on the previous stg wait
                        eng = nc.scalar if tail else nc.sync
                        eng.dma_start(out=ctxT_sb[64:128, dt, qsl], in_=stg)

                def emit_outproj(tts, tail=False):
                    for n, tt in enumerate(tts):
                        # at the tail the score tiles are dead: rotate po
                        # through their banks so nothing waits on evacuation
                        ptag = ("po", "sc0", "sc1")[n % 3] if tail else "po"
                        po = ps2.tile([128, D], F32, tag=ptag, name=ptag, bufs=1)
                        for nchk in range(2):
                            for k in range(2):
                                nc.tensor.matmul(
                                    po[:, 512 * nchk : 512 * nchk + 512],
                                    ctxT_sb[:, k, 128 * tt : 128 * tt + 128],
                                    wo_sb[:, k, 512 * nchk : 512 * nchk + 512],
                                    start=(k == 0), stop=(k == 1))
                        osb = epip.tile([128, D], BF16, tag="osb", bufs=3)
                        if tail:  # split the evacuation DVE/ACT at the tail
                            nc.vector.tensor_copy(osb[:, 0:512], po[:, 0:512])
                            nc.scalar.copy(out=osb[:, 512:], in_=po[:, 512:])
                        else:
                            nc.vector.tensor_copy(osb, po)
                        nc.scalar.dma_start(out=out[128 * tt : 128 * tt + 128, :], in_=osb)

                pending = []  # t-tiles whose output projection is deferred
                for j in (1, 0, 2, 3):
                    nst = 4 * j + 4
                    npairs = nst // 2
                    # diagonal pairs first: their exp + causal mask complete
                    # while the remaining pairs' scores stream on the PE
                    order = [npairs - 1, npairs - 2] + list(range(npairs - 2))
                    last = j == 3
                    for dt in (0, 1):
                        ats = [attnp.tile([128, NTT, 512], BF16, tag=f"at{dt}{i}",
                                          name=f"at{dt}{i}", bufs=1) for i in range(2)]
                        scs = [ps2.tile([128, 1024], F32, tag=f"sc{i}",
                                        name=f"sc{i}", bufs=1) for i in range(2)]
                        for i, p2 in enumerate(order):
                            emit_score_pair(j, dt, p2, ats, scs, npairs)
                            if p2 == npairs - 1:
                                for hh in range(2):
                                    emit_mask(j, ats[hh], (2, 3))
                            elif p2 == npairs - 2:
                                for hh in range(2):
                                    emit_mask(j, ats[hh], (0, 1))
                            # deferred output projection of the previous chunk
                            # fills the PE stall while exp gates the scs WAR
                            if i == 1 and pending:
                                emit_outproj(pending[:2])
                                pending = pending[2:]
                        if dt == 1 and pending:
                            emit_outproj(pending)
                            pending = []
                        # PV consumes s-tiles in exp-completion order so the
                        # chains end right after the last exp lands (the
                        # scheduler interleaves both chains against the ACT
                        # stream). Normalize emitted per chain. At the tail,
                        # hh=1 first so the last normalize writes ctxT
                        # directly (no stg DMA on the critical path).
                        # st 4j first: it is full-width, so the chain's
                        # start=True initializes every PSUM column (diagonal
                        # tiles only cover columns >= 128c); end on pair P-3's
                        # tiles, whose exps land last.
                        st_order = list(range(4 * j, nst)) + list(range(4 * j))
                        tail_dt = last and dt == 1
                        hh_order = (1, 0) if tail_dt else (0, 1)
                        for hh in hh_order:
                            ct = emit_pv(j, dt, hh, ats[hh], nst, st_order)
                            emit_normalize(j, dt, hh, ct, tail=tail_dt)
                    pending = list(range(4 * j, 4 * j + 4))
                emit_outproj(pending, tail=True)

    nc.compile()
    return nc


def _make_tables():
    # RoPE pair layout inside each 64-row head block: row r -> quadrant
    # q2 = r//32, half = (r%32)//16 (0: x1, 1: x2), pair ii = q2*16 + r%16.
    # Pair partner = r ^ 16 (stream_shuffle reach, within a 32-quadrant).
    r_ = np.arange(64)
    q2, rr = r_ // 32, r_ % 32
    half, ii = rr // 16, q2 * 16 + (rr % 16)
    theta = 10000.0 ** (np.arange(0, DK, 2, dtype=np.float32) / DK)  # [32]
    pos = np.arange(T, dtype=np.float32)
    ang = pos[None, :] / theta[:, None]  # [32, T]
    cc64 = np.cos(ang)[ii]                                     # [64, T]
    ss64 = np.sin(ang)[ii] * np.where(half == 0, -1.0, 1.0)[:, None]
    import ml_dtypes
    cc = np.tile(cc64, (2, 1)).astype(ml_dtypes.bfloat16)      # [128, T]
    ss = np.tile(ss64, (2, 1)).astype(ml_dtypes.bfloat16)
    m01 = (np.arange(128)[:, None] <= np.arange(128)[None, :]).astype(ml_dtypes.bfloat16)
    perm = np.where(half == 0, 2 * ii, 2 * ii + 1)             # row -> orig q/k dim
    return cc, ss, m01, perm


def _make_in_maps(x, wq, bq, wk, bk, wv, bv, wo):
    cc, ss, m01, p = _make_tables()
    in_maps = []
    for core in range(NCORES):
        b, g = divmod(core, G)
        heads = np.arange(4 * g, 4 * g + 4)
        rows_qk = np.concatenate([64 * h + p for h in heads])
        rows_v = np.concatenate([64 * h + np.arange(DK) for h in heads])
        # bias row [1, 512]: q-dt0[128], q-dt1[128], k-dt0[128], k-dt1[128]
        bqkt = np.concatenate([bq[rows_qk], bk[rows_qk]])[None, :]
        import ml_dtypes
        bf = ml_dtypes.bfloat16
        def wtile(w):  # [D, DSH] -> [128, KT*DSH] matching sbuf [p, k, n]
            return np.ascontiguousarray(
                w.reshape(KT, 128, DSH).transpose(1, 0, 2).reshape(128, KT * DSH))
        woTl = wo[:, rows_v].T.astype(np.float32)  # [DSH, D]
        woTl = woTl.reshape(2, 128, D).transpose(1, 0, 2).reshape(128, 2 * D)
        in_maps.append({
            "xT": np.ascontiguousarray(x[b].T.astype(bf)),
            "wqT": wtile(wq[rows_qk].T.astype(bf)),
            "wkT": wtile(wk[rows_qk].T.astype(bf)),
            "wvT": wtile(wv[rows_v].T.astype(bf)),
            "woT": np.ascontiguousarray(woTl),
            "bqkt": np.ascontiguousarray(bqkt.astype(np.float32)),
            "bv": np.ascontiguousarray(bv[rows_v][None, :]),
            "cc": cc, "ss": ss, "m01": m01,
            "ones": np.ones((1, 512), np.float32),
        })
    return in_maps


def _get_runner():
    """Compile once; return a jitted 8-core runner reusable across calls."""
    if "runner" in _CACHE:
        return _CACHE["runner"]
    import jax
    from jax.sharding import Mesh, PartitionSpec
    from jax.experimental.shard_map import shard_map

    install_neuronx_cc_hook()
    nc = _build_bass()

    partition_name = nc.partition_id_tensor.name if nc.partition_id_tensor else None
    in_names, out_names, out_avals = [], [], []
    for alloc in nc.m.functions[0].allocations:
        if not isinstance(alloc, mybir.MemoryLocationSet):
            continue
        name = alloc.memorylocations[0].name
        if alloc.kind == "ExternalInput":
            if name != partition_name:
                in_names.append(name)
        elif alloc.kind == "ExternalOutput":
            out_names.append(name)
            out_avals.append(
                jax.core.ShapedArray(tuple(alloc.tensor_shape), mybir.dt.np(alloc.dtype)))
    n_params = len(in_names)
    all_in = list(in_names) + list(out_names)

    def _pid():
        import jax.numpy as jnp
        from concourse.bass2jax import partition_id_tensor
        return partition_id_tensor()

    def _body(*args):
        operands = list(args)
        if partition_name is not None:
            operands.append(_pid())
        outs = _bass_exec_p.bind(
            *operands,
            out_avals=tuple(out_avals),
            in_names=tuple(all_in + ([partition_name] if partition_name else [])),
            out_names=tuple(out_names),
            lowering_input_output_aliases=(),
            sim_require_finite=True,
            sim_require_nnan=True,
            nc=nc,
        )
        return tuple(outs)

    devices = jax.devices()[:NCORES]
    mesh = Mesh(np.asarray(devices), ("core",))
    nin = n_params + len(out_names)
    sharded = jax.jit(shard_map(
        _body, mesh=mesh,
        in_specs=(PartitionSpec("core"),) * nin,
        out_specs=(PartitionSpec("core"),) * len(out_names),
        check_rep=False))

    def run(in_maps):
        concat_in = [
            np.concatenate([np.asarray(m[nm]) for m in in_maps], axis=0)
            for nm in in_names
        ]
        zeros = [np.zeros((NCORES * a.shape[0], *a.shape[1:]), a.dtype) for a in out_avals]
        out_arrs = sharded(*concat_in, *zeros)
        o = np.asarray(out_arrs[out_names.index("out")])
        return o.reshape(NCORES, T, D)

    runner = {"run": run, "sharded": sharded, "in_names": in_names,
              "out_names": out_names, "out_avals": out_avals}
    _CACHE["runner"] = runner
    return runner


def kernel(x, wq, bq, wk, bk, wv, bv, wo, bo, attn_mask):
    x = np.asarray(x, np.float32)
    in_maps = _make_in_maps(
        x, np.asarray(wq, np.float32), np.asarray(bq, np.float32),
        np.asarray(wk, np.float32), np.asarray(bk, np.float32),
        np.asarray(wv, np.float32), np.asarray(bv, np.float32),
        np.asarray(wo, np.float32))
    parts = _get_runner()["run"](in_maps)  # [8, T, D] bf16
    parts = np.asarray(parts, np.float32)
    out = parts.reshape(B, G, T, D).sum(axis=1) + np.asarray(bo, np.float32)
    return out.astype(np.float32)
